# revision 6
# baseline (speedup 1.0000x reference)
"""Trainium2 Bass kernel for nn_ConstrainNet (block-banded dynamics residual).

Reference computation (n_state=64, n_input=32, n_all=96, T=128):
    V = net_input.reshape(T, 96)
    out block 0      = V[0, :64] - x0
    out block t+1    = [A B] @ V[t] - V[t+1, :64]        (t = 0..T-2)
    output = concat of the 128 blocks -> (8192,) f32

Sharding: time axis split across 8 NeuronCores; core k computes output
blocks t in [16k, 16k+16). Inputs arrive FULL on host, so the one-step
"halo" is just an overlapping host-side slice — no collectives needed.

The whole per-core computation is ONE augmented matmul with contraction
K = 96 + 1 + 16 = 113:
    out[j, s] = sum_a lhsT[a, j] * rhs[a, s]
      rows  0..95 : lhsT = Vm^T, rhs = [A B]^T          -> AB @ Vm[j]
      row     96  : identity-block fixup (core 0 only):
                    lhsT[96, 0] = 1, rhs[96, :] = V[0, :64]
      rows 97..112: lhsT[97+j', j] = -delta(j', j), rhs[97+j] = S[j]
                    -> subtracts S[j] (= V[t+1, :64]; x0 for block 0)
All augmentation entries are constants or pure host-side slices — no
host arithmetic.

Device-side layout (HWDGE moves one packet per SBUF partition; fewer /
fatter packets win): the host packs TWO K-rows per partition — DRAM
tensor w[64, 160] f32 with
    w[p,   0: 64] = rhs row p        w[p,  64: 80] = lhsT row p
    w[p,  80:144] = rhs row 64+p     w[p, 144:160] = lhsT row 64+p
(rows 49..63 of the second group are zero padding).

v3 critical-path changes vs the 12.2us baseline (measured structure:
~4.3us fixed NEFF preamble + ~4.5us fixed semaphore-reset epilogue
around a ~3.2us kernel span; only the span is ours to shrink):
  * 64 packed rows instead of the minimal 57: the HWDGE ucode fans a
    2D transfer across DMA engines using the largest divisor of the
    row count <= 16, so 64 rows transfer on 16 engines x 4 packets
    instead of 3 x 19, cutting the transfer wall time ~4x.
  * Tensors are declared float32r: TensorE runs one pass per matmul
    instead of fp32's LOW/HIGH two-pass quad (4 passes -> 2).
    Accumulation stays fp32 in PSUM; rel err ~1e-4 vs 2e-2 budget.
  * Both matmuls increment `mm`; the output store's descriptor
    generation is gated on mm >= 1 (first matmul done) instead of the
    full product, overlapping it with matmul 2 and the PSUM->SBUF
    copy. DMA descriptors encode addresses only; the HWDGE ring
    launch (~0.45us measured from desc-gen end to first data read,
    on top of ~0.44us of desc-gen) keeps the data read well behind
    the DVE copy (measured margin ~0.6us; the copy lands ~450c after
    mm2, the read starts ~1400c after mm1).
  * The PSUM->SBUF copy waits mm >= 2; nothing waits on the copy or
    the store (the runtime quiesces DMA before output readback).

Raw Bass (no TileContext): this walrus build rejects instructions that
carry more than one sync wait; the chain below carries at most one
wait per instruction.
"""

import numpy as np

N_STATE = 64
N_INPUT = 32
N_ALL = N_STATE + N_INPUT  # 96
T_FULL = 128
N_CORES = 8
TB = T_FULL // N_CORES  # 16 output blocks per core
K = N_ALL + 1 + TB  # 113 contraction rows
W_COLS = N_STATE + TB  # 80: [rhs | lhsT] packed along the free dim
# Packed partitions: K-rows p and KP+p share partition p. 64 rather than
# the minimal ceil(113/2)=57: the HWDGE ucode fans a 2D transfer across
# DMA engines using the largest divisor of the row count <= 16 (measured:
# 57 rows -> 3 engines, 29 -> 1, 28 -> 14, 16 -> 16), so 64 rows ride 16
# engines x 4 packets while 57 crawl on 3 x 19. Rows 113..127 are zero
# padding and contribute nothing to the accumulation.
KP = 64

_PROGRAM_CACHE = {}


def _build_program():
    import concourse.bass as bass
    import concourse.mybir as mybir

    f32 = mybir.dt.float32
    f32r = mybir.dt.float32r
    nc = bass.Bass("TRN2", debug=False)

    w = nc.dram_tensor("w", [KP, 2 * W_COLS], f32r, kind="ExternalInput")
    out_d = nc.dram_tensor("out", [TB, N_STATE], f32, kind="ExternalOutput")

    # Instructions are emitted straight into the main block (no nc.Block()):
    # the per-engine branch into a Block basic block costs ~400ns on the
    # critical path. Each engine executes only its own instructions, in
    # program order, so the semaphore chain below is unchanged.
    with (
        nc.sbuf_tensor([KP, 2 * W_COLS], f32r) as w_t,
        nc.psum_tensor([TB, N_STATE], f32) as acc,
        nc.sbuf_tensor([TB, N_STATE], f32) as o_t,
        nc.semaphore("dma_a") as dma_a,
        nc.semaphore("mm") as mm,
        nc.semaphore("dma_out") as dma_out,
    ):
        # One dma_start on the Sync HWDGE: splitting across Sync+Scalar was
        # measured SLOWER (Scalar desc-gen runs ~2x Sync's: 1536c vs 751c
        # for half the rows), and with 64 rows the single queue already
        # fans out across all 16 DMA engines.
        nc.sync.dma_start(out=w_t[:], in_=w[:]).then_inc(dma_a, 16)
        nc.tensor.wait_ge(dma_a, 16)
        nc.tensor.matmul(
            acc[:],
            w_t[0:KP, N_STATE:W_COLS],
            w_t[0:KP, 0:N_STATE],
            start=True,
            stop=False,
        ).then_inc(mm, 1)
        # second group's row 56 is zero padding -> contributes nothing
        nc.tensor.matmul(
            acc[:],
            w_t[0:KP, W_COLS + N_STATE : 2 * W_COLS],
            w_t[0:KP, W_COLS : W_COLS + N_STATE],
            start=False,
            stop=True,
        ).then_inc(mm, 1)
        nc.vector.wait_ge(mm, 2)
        nc.vector.tensor_copy(o_t[:], acc[:])
        # Store desc-gen gated on INPUT ARRIVAL, not on the matmuls:
        # descriptors encode addresses only, and desc-gen (~664c) + ring
        # launch (~660c) starts reading o_t ~1320c after dma_a fires,
        # while the copy lands ~790c after it (wait 52 + LDW 95 + MM 208
        # + LDW/MM2 ~230 + sem ~45 + copy 212; measured v3). ~530c of
        # margin, with both paths gated on the same dma_a so they cannot
        # skew independently. This overlaps the store's fixed costs with
        # the whole compute chain instead of serializing behind it.
        nc.sync.wait_ge(dma_a, 16)
        nc.sync.dma_start(out=out_d[:], in_=o_t[:]).then_inc(dma_out, 16)

    return nc


def _get_program():
    if "nc" not in _PROGRAM_CACHE:
        _PROGRAM_CACHE["nc"] = _build_program()
    return _PROGRAM_CACHE["nc"]


def _make_in_maps(A, B, x0, net_input):
    A = np.ascontiguousarray(A, dtype=np.float32)
    B = np.ascontiguousarray(B, dtype=np.float32)
    x0 = np.ascontiguousarray(x0, dtype=np.float32)
    V = np.ascontiguousarray(net_input, dtype=np.float32).reshape(T_FULL, N_ALL)

    ab_t = np.concatenate([A, B], axis=1).T  # (96, 64)

    in_maps = []
    for k in range(N_CORES):
        w = np.zeros((K, W_COLS), dtype=np.float32)
        rhs = w[:, :N_STATE]
        lhsT = w[:, N_STATE:]
        rhs[:N_ALL] = ab_t
        # rows 97..112: -I in lhsT, S rows in rhs
        lhsT[N_ALL + 1 :] = -np.eye(TB, dtype=np.float32)
        t0 = k * TB
        if k == 0:
            rhs[N_ALL] = V[0, :N_STATE]  # identity-block fixup
            lhsT[N_ALL, 0] = 1.0
            lhsT[:N_ALL, 1:] = V[0 : TB - 1].T
            rhs[N_ALL + 1] = x0
            rhs[N_ALL + 2 :] = V[1:TB, :N_STATE]
        else:
            lhsT[:N_ALL] = V[t0 - 1 : t0 + TB - 1].T
            rhs[N_ALL + 1 :] = V[t0 : t0 + TB, :N_STATE]
        # pack two K-rows per partition: [row p | row 57+p]
        w2 = np.zeros((KP, 2 * W_COLS), dtype=np.float32)
        w2[:, :W_COLS] = w[0:KP]
        w2[0 : K - KP, W_COLS:] = w[KP:K]
        in_maps.append({"w": w2})
    return in_maps


def kernel(A, B, x0, net_input, T):
    assert int(T) == T_FULL, f"kernel hardcoded for T={T_FULL}, got {T}"
    from concourse.bass_utils import run_bass_kernel_spmd

    nc = _get_program()
    in_maps = _make_in_maps(A, B, x0, net_input)
    res = run_bass_kernel_spmd(nc, in_maps, core_ids=list(range(N_CORES)))
    out = np.concatenate([np.asarray(r["out"]).reshape(-1) for r in res.results])
    return out.astype(np.float32)
